# revision 11
# baseline (speedup 1.0000x reference)
"""Bidirectional cross-modal Mamba2 (SSD) scan on 8 TRN2 NeuronCores.

Sharding: core = (direction, batch): cores 0-3 run the forward stack for
batches 0-3, cores 4-7 the backward stack (host-reversed input). Each core
runs the full 4-layer Mamba2 stack on its (4096, 192) sequence, the pairs
exchange final states with an AllGather, and each core computes the gated
merge for half the tokens.

Self-contained: hardcodes all shapes from the problem spec.
"""
import contextlib

import numpy as np

import concourse.bass as bass
import concourse.mybir as mybir
import concourse.tile as tile
from concourse import bacc
from concourse.bass_utils import run_bass_kernel_spmd

F32 = mybir.dt.float32
F32R = mybir.dt.float32r
AF = mybir.ActivationFunctionType
ALU = mybir.AluOpType

DIM, NL, NSTATE, Q, P = 192, 4, 64, 128, 64
DIN, H, KC, CONV = 384, 6, 4, 512
T, NCH = 4096, 32
NPASS, PW = 8, 512  # token passes for the big GEMMs

DEBUG_LAYER_OUT = False


def _ap(t, dims, offset=0):
    b = t if isinstance(t, bass.AP) else t[:]
    return bass.AP(tensor=b.tensor, offset=b.offset + offset, ap=dims)


def build_nc():
    nc = bacc.Bacc("TRN2", target_bir_lowering=False, debug=False, num_devices=8)

    # ---------------- inputs ----------------
    u0 = nc.dram_tensor("u0", [T, DIM], F32, kind="ExternalInput")
    wz = nc.dram_tensor("wz", [NL, DIM, DIN], F32, kind="ExternalInput")
    wdt = nc.dram_tensor("wdt", [NL, DIM, H], F32, kind="ExternalInput")
    wtap = nc.dram_tensor("wtap", [NL, KC, DIM, CONV], F32, kind="ExternalInput")
    wout = nc.dram_tensor("wout", [NL, DIN, 256], F32, kind="ExternalInput")
    bz = nc.dram_tensor("bz", [NL, DIN], F32, kind="ExternalInput")
    bconv = nc.dram_tensor("bconv", [NL, CONV], F32, kind="ExternalInput")
    bdt = nc.dram_tensor("bdt", [NL, H], F32, kind="ExternalInput")
    a_in = nc.dram_tensor("a_in", [NL, H], F32, kind="ExternalInput")
    dbc = nc.dram_tensor("dbc", [NL, DIN], F32, kind="ExternalInput")
    lnpad = nc.dram_tensor("lnpad", [NL, DIM], F32, kind="ExternalInput")
    flnw = nc.dram_tensor("flnw", [1, DIM], F32, kind="ExternalInput")
    flnb = nc.dram_tensor("flnb", [1, DIM], F32, kind="ExternalInput")
    w1p = nc.dram_tensor("w1p", [DIM, 256], F32, kind="ExternalInput")
    w2p = nc.dram_tensor("w2p", [DIM, 256], F32, kind="ExternalInput")
    bgp = nc.dram_tensor("bgp", [1, 256], F32, kind="ExternalInput")
    jsel0 = nc.dram_tensor("jsel0", [Q, Q], F32, kind="ExternalInput")
    jsel1 = nc.dram_tensor("jsel1", [Q, Q], F32, kind="ExternalInput")
    ident6 = nc.dram_tensor("ident6", [Q, H * Q], F32, kind="ExternalInput")
    id128 = nc.dram_tensor("id128", [Q, Q], F32, kind="ExternalInput")

    # ---------------- outputs ----------------
    mh = nc.dram_tensor("mh", [T // 2, DIM], F32, kind="ExternalOutput")
    if DEBUG_LAYER_OUT:
        udbg = nc.dram_tensor("udbg", [NL, Q, NCH * DIM], F32, kind="ExternalOutput")

    # internal DRAM
    ag_in = nc.dram_tensor("ag_in", [T, DIM], F32)
    ag_out = nc.dram_tensor("ag_out", [2, T, DIM], F32)
    x_dr = nc.dram_tensor("x_dr", [3, Q, T], F32)
    sz_dr = nc.dram_tensor("sz_dr", [3, Q, T], F32)

    with tile.TileContext(nc) as tc, contextlib.ExitStack() as stk:
        pers = stk.enter_context(tc.tile_pool(name="pers", bufs=1))
        cks = stk.enter_context(tc.tile_pool(name="cks", bufs=2))
        stg = stk.enter_context(tc.tile_pool(name="stg", bufs=4))
        ckio = stk.enter_context(tc.tile_pool(name="ckio", bufs=2))

        # ---- persistent tiles ----
        u_T = pers.tile([Q, NCH * DIM], F32)          # token-major residual
        V_sb = pers.tile([NSTATE, H * P], F32)        # running chunk state
        id_sb = pers.tile([Q, Q], F32)
        i6_sb = pers.tile([Q, H * Q], F32)
        j0_sb = pers.tile([Q, Q], F32)
        j1_sb = pers.tile([Q, Q], F32)
        eps_t = pers.tile([Q, 1], F32)
        eps1 = pers.tile([1, 1], F32)
        ones_col = pers.tile([Q, 1], F32)
        ones_r = pers.tile([1, Q], F32R)
        fw_bc = pers.tile([Q, DIM], F32)
        fb_bc = pers.tile([Q, DIM], F32)
        w1_sb = pers.tile([Q, 2, 256], F32R)          # [slab(128|64), 256]
        w2_sb = pers.tile([Q, 2, 256], F32R)
        bgp_sb = pers.tile([1, 256], F32R)

        # initial residual load: u_T[p, (c, d)] = u0[c*128+p, d]
        nc.sync.dma_start(
            _ap(u_T, [[NCH * DIM, Q], [DIM, NCH], [1, DIM]]),
            _ap(u0, [[DIM, Q], [Q * DIM, NCH], [1, DIM]]))
        nc.sync.dma_start(id_sb[:], id128[:])
        nc.sync.dma_start(i6_sb[:], ident6[:])
        nc.sync.dma_start(j0_sb[:], jsel0[:])
        nc.sync.dma_start(j1_sb[:], jsel1[:])
        nc.vector.memset(eps_t[:], 1e-5)
        nc.vector.memset(eps1[:], 1e-5)
        nc.vector.memset(ones_col[:], 1.0)
        ones_f = pers.tile([1, Q], F32)
        nc.vector.memset(ones_f[:], 1.0)
        nc.vector.tensor_copy(ones_r[:], ones_f[:])
        tmpw = pers.tile([1, DIM], F32)
        nc.sync.dma_start(tmpw[:], flnw[:])
        nc.gpsimd.partition_broadcast(fw_bc[:], tmpw[:])
        tmpb = pers.tile([1, DIM], F32)
        nc.sync.dma_start(tmpb[:], flnb[:])
        nc.gpsimd.partition_broadcast(fb_bc[:], tmpb[:])
        nc.gpsimd.dma_start(w1_sb[:, 0, :], w1p[0:Q, :])
        nc.gpsimd.dma_start(w1_sb[:64, 1, :], w1p[Q:DIM, :])
        nc.gpsimd.dma_start(w2_sb[:, 0, :], w2p[0:Q, :])
        nc.gpsimd.dma_start(w2_sb[:64, 1, :], w2p[Q:DIM, :])
        nc.gpsimd.dma_start(bgp_sb[:], bgp[:])
        nc.vector.memset(V_sb[:], 0.0)

        def emit_ln(uc, out_ap):
            st = cks.tile([Q, 6], F32, name="st", tag="st")
            nc.vector.bn_stats(st[:], uc)
            mv = cks.tile([Q, 2], F32, name="mv", tag="mv")
            nc.vector.bn_aggr(mv[:], st[:])
            sd = cks.tile([Q, 1], F32, name="sd", tag="sd")
            nc.scalar.activation(sd[:], mv[:, 1:2], AF.Ln, bias=eps_t[:])
            rstd = cks.tile([Q, 1], F32, name="rstd", tag="rstd")
            nc.scalar.activation(rstd[:], sd[:], AF.Exp, scale=-0.5)
            nc.vector.tensor_scalar(out=out_ap, in0=uc, scalar1=mv[:, 0:1],
                                    scalar2=rstd[:], op0=ALU.subtract, op1=ALU.mult)

        for l in range(NL):
            with tc.tile_pool(name="dtp", bufs=1) as dtp, \
                 tc.tile_pool(name="wsm", bufs=1) as wsm, \
                 tc.tile_pool(name="bcp", bufs=1) as bcp:
                dt_sp = dtp.tile([H, T], F32)
                dA = dtp.tile([H, T + 4], F32)
                Bd = bcp.tile([64, T], F32)
                Cd = bcp.tile([64, T], F32)
                wo_sb = [wsm.tile([Q, 256], F32R, name=f"wo{m}") for m in range(3)]
                for m in range(3):
                    nc.gpsimd.dma_start(wo_sb[m][:], wout[l, m * Q:(m + 1) * Q, :])
                bdt_sb = wsm.tile([H, 1], F32)
                nc.sync.dma_start(bdt_sb[:], _ap(bdt, [[1, H], [1, 1]], l * H))
                a_sb = wsm.tile([H, 1], F32)
                nc.sync.dma_start(a_sb[:], _ap(a_in, [[1, H], [1, 1]], l * H))
                db_sb = [wsm.tile([Q, 1], F32, name=f"db{m}") for m in range(3)]
                for m in range(3):
                    nc.sync.dma_start(db_sb[m][:], _ap(dbc, [[1, Q], [1, 1]], l * DIN + m * Q))

                with tc.tile_pool(name="wbig", bufs=1) as wbig, \
                     tc.tile_pool(name="lnp", bufs=1) as lnp:
                    wt_sb = [[wbig.tile([Q, CONV], F32R, name=f"wt{k}0"),
                              wbig.tile([64, CONV], F32R, name=f"wt{k}1")]
                             for k in range(KC)]
                    for k in range(KC):
                        nc.gpsimd.dma_start(wt_sb[k][0][:], wtap[l, k, 0:Q, :])
                        nc.gpsimd.dma_start(wt_sb[k][1][:], wtap[l, k, Q:DIM, :])
                    wz_sb = [wbig.tile([Q, DIN], F32R, name="wz0"),
                             wbig.tile([64, DIN], F32R, name="wz1")]
                    nc.gpsimd.dma_start(wz_sb[0][:], wz[l, 0:Q, :])
                    nc.gpsimd.dma_start(wz_sb[1][:], wz[l, Q:DIM, :])
                    wdt_sb = [wbig.tile([Q, H], F32R, name="wdt0"),
                              wbig.tile([64, H], F32R, name="wdt1")]
                    nc.gpsimd.dma_start(wdt_sb[0][:], wdt[l, 0:Q, :])
                    nc.gpsimd.dma_start(wdt_sb[1][:], wdt[l, Q:DIM, :])
                    bz_sb = [wbig.tile([Q, 1], F32, name=f"bz{m}") for m in range(3)]
                    for m in range(3):
                        nc.sync.dma_start(bz_sb[m][:], _ap(bz, [[1, Q], [1, 1]], l * DIN + m * Q))
                    bcv_sb = [wbig.tile([Q, 1], F32, name=f"bcv{m}") for m in range(3)]
                    for m in range(3):
                        nc.sync.dma_start(bcv_sb[m][:], _ap(bconv, [[1, Q], [1, 1]], l * CONV + m * Q))
                    bcB_sb = wbig.tile([64, 1], F32)
                    nc.sync.dma_start(bcB_sb[:], _ap(bconv, [[1, 64], [1, 1]], l * CONV + DIN))
                    bcC_sb = wbig.tile([64, 1], F32)
                    nc.sync.dma_start(bcC_sb[:], _ap(bconv, [[1, 64], [1, 1]], l * CONV + DIN + 64))

                    lnD = [lnp.tile([Q, 4 + T], F32R, name="lnD0"),
                           lnp.tile([64, 4 + T], F32R, name="lnD1")]
                    for pcol in range(3):
                        nc.gpsimd.dma_start(lnD[0][:, pcol:pcol + 1],
                                            _ap(lnpad, [[1, Q], [1, 1]], l * DIM))
                        nc.gpsimd.dma_start(lnD[1][:, pcol:pcol + 1],
                                            _ap(lnpad, [[1, 64], [1, 1]], l * DIM + Q))

                    # ---- phase 1: LN + transpose into lnD ----
                    with tc.tile_pool(name="ps1", bufs=2, space="PSUM") as ps1:
                        for c in range(NCH):
                            lnc = cks.tile([Q, DIM], F32, name="lnc", tag="lnc")
                            emit_ln(u_T[:, c * DIM:(c + 1) * DIM], lnc[:])
                            pt0 = ps1.tile([Q, Q], F32, name="pt0", tag="pt0")
                            nc.tensor.transpose(pt0[:], lnc[:, 0:Q], id_sb[:])
                            pt1 = ps1.tile([64, Q], F32, name="pt1", tag="pt1")
                            nc.tensor.transpose(pt1[:], lnc[:, Q:DIM], id_sb[:])
                            nc.scalar.copy(out=lnD[0][:, 3 + c * Q:3 + (c + 1) * Q], in_=pt0[:])
                            nc.vector.tensor_copy(lnD[1][:, 3 + c * Q:3 + (c + 1) * Q], pt1[:])

                    # ---- phase 2: big GEMMs over token passes ----
                    with tc.tile_pool(name="ps2", bufs=4, space="PSUM") as ps2:
                        for n in range(NPASS):
                            t0, t1 = n * PW, (n + 1) * PW
                            for m in range(3):  # conv x-part
                                pc = ps2.tile([Q, PW], F32, name="pg", tag="pg")
                                first = True
                                for s in range(2):
                                    for k in range(KC):
                                        nc.tensor.matmul(
                                            pc[:], wt_sb[k][s][:, m * Q:(m + 1) * Q],
                                            lnD[s][:, t0 + k:t0 + k + PW],
                                            start=first, stop=(s == 1 and k == KC - 1))
                                        first = False
                                xs = stg.tile([Q, PW], F32, name="xs", tag="stg")
                                nc.scalar.activation(xs[:], pc[:], AF.Silu,
                                                     bias=bcv_sb[m][:])
                                nc.sync.dma_start(x_dr[m, :, t0:t1], xs[:])
                            for bc_i, (dst, bias_t) in enumerate(((Bd, bcB_sb), (Cd, bcC_sb))):
                                pcb = ps2.tile([64, PW], F32, name="pgBC", tag="pg")
                                first = True
                                for s in range(2):
                                    for k in range(KC):
                                        nc.tensor.matmul(
                                            pcb[:],
                                            wt_sb[k][s][:, DIN + bc_i * 64:DIN + (bc_i + 1) * 64],
                                            lnD[s][:, t0 + k:t0 + k + PW],
                                            start=first, stop=(s == 1 and k == KC - 1))
                                        first = False
                                nc.scalar.activation(dst[:, t0:t1], pcb[:], AF.Silu,
                                                     bias=bias_t[:])
                            for m in range(3):  # z-part
                                pz = ps2.tile([Q, PW], F32, name="pg2", tag="pg")
                                for s in range(2):
                                    nc.tensor.matmul(pz[:], wz_sb[s][:, m * Q:(m + 1) * Q],
                                                     lnD[s][:, 3 + t0:3 + t1],
                                                     start=(s == 0), stop=(s == 1))
                                zs = stg.tile([Q, PW], F32, name="zs", tag="stg")
                                nc.scalar.activation(zs[:], pz[:], AF.Silu,
                                                     bias=bz_sb[m][:])
                                nc.sync.dma_start(sz_dr[m, :, t0:t1], zs[:])
                            pdt = ps2.tile([H, PW], F32, name="pdt", tag="pg")
                            for s in range(2):
                                nc.tensor.matmul(pdt[:], wdt_sb[s][:], lnD[s][:, 3 + t0:3 + t1],
                                                 start=(s == 0), stop=(s == 1))
                            nc.vector.tensor_scalar_add(dt_sp[:, t0:t1], pdt[:],
                                                        bdt_sb[:])

                # ---- phase 3: dt-derived sequences + SSD chunk loop ----
                with tc.tile_pool(name="ckp", bufs=1) as ckp, \
                     tc.tile_pool(name="ckp2", bufs=2) as ckp2, \
                     tc.tile_pool(name="ps3", bufs=1, space="PSUM") as ps3, \
                     tc.tile_pool(name="ps3b", bufs=2, space="PSUM") as ps3b:
                    A_cs = ckp.tile([H, T], F32)
                    edA = ckp.tile([H, T + 1], F32)
                    nc.vector.memset(dA[:, T:T + 4], 0.0)
                    nc.scalar.activation(dA[:, 0:T], dt_sp[:], AF.Exp)
                    nc.scalar.activation(dt_sp[:], dA[:, 0:T], AF.Ln, bias=1.0)
                    nc.vector.tensor_scalar_mul(dA[:, 0:T], dt_sp[:], a_sb[:])
                    nc.vector.tensor_tensor_scan(out=A_cs[:], data0=dA[:, 0:T],
                                                 data1=dA[:, 0:T], initial=0.0,
                                                 op0=ALU.add, op1=ALU.bypass)
                    nc.scalar.activation(edA[:], dA[:, 0:T + 1], AF.Exp)
                    nc.vector.memset(_ap(edA, [[T + 1, H], [Q, NCH]]), 0.0)

                    for c in range(NCH):
                        c0, c1 = c * Q, (c + 1) * Q
                        lastc, nxt = c0 + Q - 1, c1
                        x_ck = [ckio.tile([Q, Q], F32, name=f"xck{m}", tag=f"xck{m}")
                                for m in range(3)]
                        for m in range(3):
                            nc.sync.dma_start(x_ck[m][:], x_dr[m, :, c0:c1])
                        sz_ck = [ckio.tile([Q, Q], F32, name=f"szck{m}", tag=f"szck{m}")
                                 for m in range(3)]
                        for m in range(3):
                            nc.sync.dma_start(sz_ck[m][:], sz_dr[m, :, c0:c1])
                        dsa = cks.tile([H, Q], F32, name="dsa", tag="dsa")
                        nc.vector.tensor_scalar(out=dsa[:], in0=A_cs[:, c0:c1],
                                                scalar1=A_cs[:, lastc:lastc + 1],
                                                scalar2=dA[:, nxt:nxt + 1],
                                                op0=ALU.subtract, op1=ALU.subtract)
                        dse = cks.tile([H, Q], F32, name="dse", tag="dse")
                        nc.scalar.activation(dse[:], dsa[:], AF.Exp, scale=-1.0)
                        cda = cks.tile([H, 1], F32, name="cda", tag="cda")
                        nc.vector.tensor_scalar(out=cda[:], in0=A_cs[:, lastc:lastc + 1],
                                                scalar1=A_cs[:, c0:c0 + 1],
                                                scalar2=dA[:, nxt:nxt + 1],
                                                op0=ALU.subtract, op1=ALU.add)
                        cde = cks.tile([H, 1], F32, name="cde", tag="cde")
                        nc.scalar.activation(cde[:], cda[:], AF.Exp)
                        ptd = ps3b.tile([Q, 12], F32, name="ptd", tag="s3")
                        nc.tensor.transpose(ptd[:, 0:6], dt_sp[:, c0:c1], id_sb[0:H, 0:H])
                        nc.tensor.transpose(ptd[:, 6:12], dse[:], id_sb[0:H, 0:H])
                        dtdsT = cks.tile([Q, 12], F32, name="dtdsT", tag="dtdsT")
                        nc.scalar.copy(out=dtdsT[:], in_=ptd[:])
                        pcd = ps3b.tile([1, H], F32, name="pcd", tag="s3")
                        nc.tensor.transpose(pcd[:], cde[:], id_sb[0:H, 0:H])
                        cdr = cks.tile([1, H], F32, name="cdr", tag="cdr")
                        nc.vector.tensor_copy(cdr[:], pcd[:])
                        cdbc = cks.tile([64, H], F32, name="cdbc", tag="cdbc")
                        nc.gpsimd.partition_broadcast(cdbc[:], cdr[:])
                        eflat = cks.tile([1, H * Q], F32, name="eflat", tag="eflat")
                        nc.sync.dma_start(eflat[:], edA[:, c0:c1])
                        ld0 = ckp2.tile([Q, H * Q], F32, name="ld0", tag="ld0", bufs=1)
                        nc.gpsimd.partition_broadcast(ld0[:], eflat[:])
                        Lsb = ckp2.tile([Q, H * Q], F32, name="Lsb", tag="Lsb")
                        nc.vector.tensor_tensor_scan(out=Lsb[:], data0=ld0[:],
                                                     data1=i6_sb[:], initial=0.0,
                                                     op0=ALU.mult, op1=ALU.add)
                        ebc = ckp2.tile([64, H * Q], F32, name="ebc", tag="ebc", bufs=1)
                        nc.gpsimd.partition_broadcast(ebc[:], Lsb[0:1, :])
                        cs_t = ckp2.tile([64, H * Q], F32, name="cs", tag="cs")
                        nc.vector.tensor_tensor(
                            out=_ap(cs_t, [[H * Q, 64], [Q, H], [1, Q]]),
                            in0=_ap(Cd, [[T, 64], [0, H], [1, Q]], c0),
                            in1=_ap(ebc, [[H * Q, 64], [Q, H], [1, Q]]),
                            op=ALU.mult)
                        pgt = ps3.tile([Q, H * Q], F32, name="pgt", tag="pgt")
                        for h in range(H):
                            nc.tensor.matmul(pgt[:, h * Q:(h + 1) * Q], Bd[:, c0:c1],
                                             Cd[:, c0:c1], start=True, stop=True)
                        mt = Lsb
                        nc.vector.tensor_tensor(out=mt[:], in0=pgt[:], in1=Lsb[:],
                                                op=ALU.mult)
                        pxt = ps3.tile([Q, H * P], F32, name="pxt", tag="pxt")
                        for m in range(3):
                            nc.tensor.transpose(pxt[:, m * Q:(m + 1) * Q],
                                                x_ck[m][:], id_sb[:])
                        xt = ckp2.tile([Q, H * P], F32, name="xt", tag="xt")
                        nc.vector.tensor_tensor(
                            out=_ap(xt, [[H * P, Q], [P, H], [1, P]]),
                            in0=_ap(pxt, [[H * P, Q], [P, H], [1, P]]),
                            in1=_ap(dtdsT, [[12, Q], [1, H], [0, P]]),
                            op=ALU.mult)
                        pbt = ps3b.tile([Q, 64], F32, name="pbt", tag="s3")
                        nc.tensor.transpose(pbt[:], Bd[:, c0:c1], id_sb[0:64, 0:64])
                        btc = cks.tile([Q, 64], F32, name="btc", tag="btc")
                        nc.scalar.copy(out=btc[:], in_=pbt[:])
                        btd = ckp2.tile([Q, H * P], F32, name="btd", tag="btd")
                        nc.vector.tensor_tensor(
                            out=_ap(btd, [[H * P, Q], [P, H], [1, P]]),
                            in0=_ap(btc, [[64, Q], [0, H], [1, P]]),
                            in1=_ap(dtdsT, [[12, Q], [1, H], [0, P]], 6),
                            op=ALU.mult)
                        pst = ps3.tile([NSTATE, H * P], F32, name="pst", tag="pst")
                        for h in range(H):
                            nc.tensor.matmul(pst[:, h * P:(h + 1) * P],
                                             btd[:, h * P:(h + 1) * P],
                                             xt[:, h * P:(h + 1) * P],
                                             start=True, stop=True)
                        py = ps3.tile([Q, 3 * Q], F32, name="py", tag="py")
                        for h in range(H):
                            m, r = h // 2, h % 2
                            psl = py[r * P:(r + 1) * P, m * Q:(m + 1) * Q]
                            nc.tensor.matmul(psl, xt[:, h * P:(h + 1) * P],
                                             mt[:, h * Q:(h + 1) * Q],
                                             start=True, stop=False)
                            nc.tensor.matmul(psl, V_sb[:, h * P:(h + 1) * P],
                                             cs_t[:, h * Q:(h + 1) * Q],
                                             start=False, stop=True)
                        vtmp = cks.tile([NSTATE, H * P], F32, name="vtmp", tag="vtmp")
                        nc.vector.tensor_tensor(
                            out=_ap(vtmp, [[H * P, NSTATE], [P, H], [1, P]]),
                            in0=_ap(V_sb, [[H * P, NSTATE], [P, H], [1, P]]),
                            in1=_ap(cdbc, [[H, NSTATE], [1, H], [0, P]]),
                            op=ALU.mult)
                        nc.vector.tensor_tensor(out=V_sb[:], in0=vtmp[:], in1=pst[:],
                                                op=ALU.add)
                        yn = ckp2.tile([Q, 3 * Q], F32R, name="yn", tag="yn")
                        pss = ps3b.tile([1, Q], F32, name="pss", tag="s3")
                        for m in range(3):
                            tm = cks.tile([Q, Q], F32, name=f"tm{m}", tag=f"tm{m}")
                            nc.vector.scalar_tensor_tensor(
                                out=tm[:], in0=x_ck[m][:], scalar=db_sb[m][:],
                                in1=py[:, m * Q:(m + 1) * Q],
                                op0=ALU.mult, op1=ALU.add)
                            nc.vector.tensor_tensor(out=yn[:, m * Q:(m + 1) * Q],
                                                    in0=tm[:], in1=sz_ck[m][:],
                                                    op=ALU.mult)
                            sq = cks.tile([Q, Q], F32, name=f"sq{m}", tag=f"sq{m}")
                            nc.vector.tensor_tensor(out=sq[:], in0=yn[:, m * Q:(m + 1) * Q],
                                                    in1=yn[:, m * Q:(m + 1) * Q],
                                                    op=ALU.mult)
                            nc.tensor.matmul(pss[:], ones_col[:], sq[:],
                                             start=(m == 0), stop=(m == 2))
                        sdr = cks.tile([1, Q], F32, name="sdr", tag="sdr")
                        nc.scalar.activation(sdr[:], pss[:], AF.Ln, bias=eps1[:],
                                             scale=1.0 / DIN)
                        rsr = cks.tile([1, Q], F32, name="rsr", tag="rsr")
                        nc.scalar.activation(rsr[:], sdr[:], AF.Exp, scale=-0.5)
                        prs = ps3b.tile([Q, 1], F32, name="prs", tag="s3")
                        nc.tensor.transpose(prs[:], rsr[:], id_sb[0:1, 0:1])
                        rsc = cks.tile([Q, 1], F32, name="rsc", tag="rsc")
                        nc.vector.tensor_copy(rsc[:], prs[:])
                        pwo = ps3b.tile([Q, 256], F32, name="pwo", tag="pwo", bufs=1)
                        for m in range(3):
                            nc.tensor.matmul(pwo[:], yn[:, m * Q:(m + 1) * Q], wo_sb[m][:],
                                             start=(m == 0), stop=(m == 2))
                        nc.vector.scalar_tensor_tensor(
                            out=u_T[:, c * DIM:(c + 1) * DIM], in0=pwo[:, 0:DIM],
                            scalar=rsc[:], in1=u_T[:, c * DIM:(c + 1) * DIM],
                            op0=ALU.mult, op1=ALU.add)
            if DEBUG_LAYER_OUT:
                nc.sync.dma_start(udbg[l], u_T[:])

        # ---- final LN + allgather ----
        for c in range(NCH):
            uc = u_T[:, c * DIM:(c + 1) * DIM]
            lnc = cks.tile([Q, DIM], F32, name="lnc2", tag="lnc")
            emit_ln(uc, lnc[:])
            t2 = cks.tile([Q, DIM], F32, name="t2f", tag="t2f")
            nc.vector.tensor_tensor(out=t2[:], in0=lnc[:], in1=fw_bc[:], op=ALU.mult)
            nc.vector.tensor_tensor(out=uc, in0=t2[:], in1=fb_bc[:], op=ALU.add)
            nc.sync.dma_start(ag_in[c * Q:(c + 1) * Q, :], uc)

        nc.gpsimd.collective_compute(
            "AllGather", ALU.bypass,
            replica_groups=[[0, 4], [1, 5], [2, 6], [3, 7]],
            ins=[ag_in[:].opt()], outs=[ag_out[:].opt()])

        # ---- merge ----
        with tc.tile_pool(name="mrg", bufs=2) as mrg, \
             tc.tile_pool(name="psm", bufs=3, space="PSUM") as psm:
            for c in range(NCH // 2):
                pc_ = NCH - 1 - c
                p0 = mrg.tile([Q, DIM], F32, name="p0", tag="p0")
                nc.gpsimd.dma_start(p0[:], ag_out[0, pc_ * Q:(pc_ + 1) * Q, :])
                p1 = mrg.tile([Q, DIM], F32, name="p1", tag="p1")
                nc.gpsimd.dma_start(p1[:], ag_out[1, pc_ * Q:(pc_ + 1) * Q, :])
                prv = psm.tile([Q, DIM], F32, name="prv", tag="prv", bufs=2)
                nc.tensor.matmul(prv[:], j0_sb[:], p0[:], start=True, stop=False)
                nc.tensor.matmul(prv[:], j1_sb[:], p1[:], start=False, stop=True)
                peer = mrg.tile([Q, DIM], F32, name="peer", tag="peer")
                nc.scalar.copy(out=peer[:], in_=prv[:])
                uc = u_T[:, c * DIM:(c + 1) * DIM]
                pot = psm.tile([Q, Q], F32, name="pot", tag="pm")
                nc.tensor.transpose(pot[:], uc[:, 0:Q], id_sb[:])
                pot1 = psm.tile([64, Q], F32, name="pot1", tag="pm")
                nc.tensor.transpose(pot1[:], uc[:, Q:DIM], id_sb[:])
                ppt = psm.tile([Q, Q], F32, name="ppt", tag="pm")
                nc.tensor.transpose(ppt[:], peer[:, 0:Q], id_sb[:])
                ppt1 = psm.tile([64, Q], F32, name="ppt1", tag="pm")
                nc.tensor.transpose(ppt1[:], peer[:, Q:DIM], id_sb[:])
                oT0 = mrg.tile([Q, Q], F32R, name="oT0", tag="oT0")
                nc.vector.tensor_copy(oT0[:], pot[:])
                oT1 = mrg.tile([64, Q], F32R, name="oT1", tag="oT1")
                nc.scalar.copy(out=oT1[:], in_=pot1[:])
                pT0 = mrg.tile([Q, Q], F32R, name="pT0", tag="pT0")
                nc.vector.tensor_copy(pT0[:], ppt[:])
                pT1 = mrg.tile([64, Q], F32R, name="pT1", tag="pT1")
                nc.scalar.copy(out=pT1[:], in_=ppt1[:])
                pg = psm.tile([Q, 256], F32, name="pgm", tag="pgm", bufs=2)
                nc.tensor.matmul(pg[:], oT0[:], w1_sb[:, 0, :], start=True, stop=False)
                nc.tensor.matmul(pg[:], oT1[:], w1_sb[0:64, 1, :], start=False, stop=False)
                nc.tensor.matmul(pg[:], pT0[:], w2_sb[:, 0, :], start=False, stop=False)
                nc.tensor.matmul(pg[:], pT1[:], w2_sb[0:64, 1, :], start=False, stop=False)
                nc.tensor.matmul(pg[:], ones_r[:], bgp_sb[:], start=False, stop=True)
                g = mrg.tile([Q, DIM], F32, name="g", tag="g")
                nc.scalar.activation(g[:], pg[:, 0:DIM], AF.Sigmoid)
                df = mrg.tile([Q, DIM], F32, name="df", tag="df")
                nc.vector.tensor_tensor(out=df[:], in0=peer[:], in1=uc, op=ALU.subtract)
                gd = mrg.tile([Q, DIM], F32, name="gd", tag="gd")
                nc.vector.tensor_tensor(out=gd[:], in0=g[:], in1=df[:], op=ALU.mult)
                mo = mrg.tile([Q, DIM], F32, name="mo", tag="mo")
                nc.vector.tensor_tensor(out=mo[:], in0=gd[:], in1=uc, op=ALU.add)
                nc.sync.dma_start(mh[c * Q:(c + 1) * Q, :], mo[:])

    nc.compile()
    return nc


def _prep_dir_params(p):
    """Host-side folding for one direction's params -> per-core input arrays."""
    out = {}
    ln_w = np.asarray(p['ln_w'], np.float32)
    ln_b = np.asarray(p['ln_b'], np.float32)
    W_in = np.asarray(p['W_in'], np.float32)
    conv_w = np.asarray(p['conv_w'], np.float32)
    conv_b = np.asarray(p['conv_b'], np.float32)
    Wg_ = W_in * ln_w[:, :, None]
    bias_proj = np.einsum('ld,ldp->lp', ln_b, W_in)
    out['wz'] = np.ascontiguousarray(Wg_[:, :, :DIN])
    out['wdt'] = np.ascontiguousarray(Wg_[:, :, DIN + CONV:])
    WxBC = Wg_[:, :, DIN:DIN + CONV]
    out['wtap'] = np.ascontiguousarray(
        np.stack([WxBC * conv_w[:, None, :, k] for k in range(KC)], 1))
    W_out = np.asarray(p['W_out'], np.float32)
    norm_w = np.asarray(p['norm_w'], np.float32)
    Wo = W_out * norm_w[:, :, None]
    out['wout'] = np.concatenate(
        [Wo, np.zeros((NL, DIN, 256 - DIM), np.float32)], -1)
    out['bz'] = np.ascontiguousarray(bias_proj[:, :DIN])
    out['bconv'] = np.ascontiguousarray(
        bias_proj[:, DIN:DIN + CONV] * conv_w.sum(-1) + conv_b)
    out['bdt'] = np.ascontiguousarray(
        bias_proj[:, DIN + CONV:] + np.asarray(p['dt_bias'], np.float32))
    out['a_in'] = -np.exp(np.asarray(p['A_log'], np.float32))
    out['dbc'] = np.repeat(np.asarray(p['D'], np.float32), P, axis=1)
    with np.errstate(divide='ignore', invalid='ignore'):
        out['lnpad'] = np.where(ln_w != 0, -ln_b / ln_w, 0.0).astype(np.float32)
    out['flnw'] = np.asarray(p['fln_w'], np.float32).reshape(1, DIM)
    out['flnb'] = np.asarray(p['fln_b'], np.float32).reshape(1, DIM)
    return out


_NC_CACHE = {}


def _get_nc():
    if 'nc' not in _NC_CACHE:
        _NC_CACHE['nc'] = build_nc()
    return _NC_CACHE['nc']


def make_in_maps(semantic, features, fwd_params, bwd_params, Wg, bg):
    semantic = np.asarray(semantic, np.float32)
    features = np.asarray(features, np.float32)
    Wg = np.asarray(Wg, np.float32)
    bg = np.asarray(bg, np.float32)
    Bn, N = semantic.shape[0], semantic.shape[1]
    pf = _prep_dir_params(fwd_params)
    pb = _prep_dir_params(bwd_params)

    def pad(w):
        return np.ascontiguousarray(
            np.concatenate([w, np.zeros((w.shape[0], 256 - DIM), np.float32)], -1))
    Wg_f, Wg_b = Wg[:DIM], Wg[DIM:]
    gate_f = dict(w1p=pad(-Wg_f), w2p=pad(-Wg_b),
                  bgp=np.concatenate([-bg, np.zeros(256 - DIM, np.float32)]).reshape(1, 256))
    gate_b = dict(w1p=pad(Wg_b), w2p=pad(Wg_f),
                  bgp=np.concatenate([bg, np.zeros(256 - DIM, np.float32)]).reshape(1, 256))
    J = np.eye(Q, dtype=np.float32)[::-1].copy()
    Z = np.zeros((Q, Q), np.float32)
    consts = dict(
        ident6=np.ascontiguousarray(np.tile(np.eye(Q, dtype=np.float32), (1, H))),
        id128=np.eye(Q, dtype=np.float32))

    in_maps = []
    for core in range(8):
        fwd = core < 4
        b = core % 4
        inter = np.stack([semantic[b], features[b]], 1).reshape(2 * N, DIM)
        u0c = inter if fwd else inter[::-1]
        m = dict(u0=np.ascontiguousarray(u0c), **(pf if fwd else pb),
                 **(gate_f if fwd else gate_b), **consts,
                 jsel0=(Z if fwd else J), jsel1=(J if fwd else Z))
        in_maps.append(m)
    return in_maps


def assemble(results, Bn, N):
    sem = np.empty((Bn, N, DIM), np.float32)
    fea = np.empty((Bn, N, DIM), np.float32)
    for b in range(Bn):
        mhf = results[b]["mh"]
        mhb = results[4 + b]["mh"]
        nat = np.concatenate([mhf, mhb[::-1]], 0)       # (4096, 192)
        r = nat.reshape(N, 2, DIM)
        sem[b], fea[b] = r[:, 0, :], r[:, 1, :]
    return sem, fea


def kernel(semantic, features, fwd_params, bwd_params, Wg, bg):
    nc = _get_nc()
    in_maps = make_in_maps(semantic, features, fwd_params, bwd_params, Wg, bg)
    res = run_bass_kernel_spmd(nc, in_maps, list(range(8)))
    Bn, N = np.asarray(semantic).shape[:2]
    return assemble(res.results, Bn, N)
